# revision 49
# baseline (speedup 1.0000x reference)
"""Trainium2 Bass kernel for nn_DiffusionOrderingNetwork (3-layer GAT, N=50000,
E=800000, softmax over nodes), SPMD across 8 NeuronCores.

Design:
- Nodes sorted by dst-edge, sharded into 8 contiguous node ranges with ~equal
  edge counts. Per-core greedy tiling: <=8 dst nodes and <=128 edges per tile.
- Per layer, each core computes per-node "records" [asrc(6)|xs(36)|adst(6)]
  via one matmul per 128-slot chunk, AllGathers them into a replicated table
  with 128-wide (256 B) rows, then fetches per-edge source records with ONE
  dma_gather per 32-tile super-block (int16 indices rebased to the table
  middle; negative indices address rows below the base).
- Layer 1 needs no gather: records take only 17 values (node types), expanded
  per edge by one-hot matmuls against a [17, 48] type-record table.
- a_dst expansion per edge via block-diagonal "stacked OHT" matmuls.
- Layer 3 (concat=False) avoids the 216-wide per-edge product by scaling the
  dst one-hots with exp(e) per head (OHx) and accumulating per-head sums.
- Final softmax over nodes via two tiny AllReduces; output [64, SLOTMAX] is
  transposed/unsharded on host.
"""
import sys
sys.path.insert(0, '/opt/trn_rl_repo')
import numpy as np
import ml_dtypes
from contextlib import ExitStack

# ======================= host prep =======================

N = 50000
E = 800000
H = 6
C1 = 6
HID = 36
D = 64
NT = 17
NEG = 0.2
NCORES = 8
EPT = 128          # edges per tile
SPT = 8            # node slots per tile
KSUP = 32          # tiles per super-block -> 256 psum cols
RW = 48            # record row: asrc(6) | xs(36) | adst(6)
RWT = 128          # table row width (256 B rows for dma_gather)


def _fold_ws(W, a):
    # ws[d, h] = sum_c W[d, h*C+c] * a[h, c]
    h, c = a.shape
    return np.einsum('dhc,hc->dh', W.reshape(W.shape[0], h, c), a).astype(np.float32)


def host_prep(x, edge_index, emb, w1, as1, ad1, b1, r1,
              w2, as2, ad2, b2, r2, w3, as3, ad3, b3, r3):
    x = np.asarray(x).astype(np.int64)
    ei = np.asarray(edge_index).astype(np.int64)
    N = x.shape[0]
    NT = emb.shape[0]
    D = emb.shape[1]
    for b in (b1, b2, b3):
        assert np.abs(np.asarray(b)).max() == 0.0, "nonzero bias breaks pad-column math"

    # --- edges with self loops, sorted by dst ---
    src = np.concatenate([ei[0], np.arange(N, dtype=np.int64)])
    dst = np.concatenate([ei[1], np.arange(N, dtype=np.int64)])
    order = np.argsort(dst, kind='stable')
    srcs = src[order]
    dsts = dst[order]
    ET = srcs.shape[0]
    deg = np.bincount(dst, minlength=N).astype(np.int64)
    assert deg.min() >= 1 and deg.max() <= EPT, deg.max()
    node_ptr = np.concatenate([[0], np.cumsum(deg)])  # edge range per node

    # --- shard nodes into NCORES contiguous ranges with ~equal edges ---
    cum = np.cumsum(deg)
    bnds = [0]
    for k in range(1, NCORES):
        bnds.append(int(np.searchsorted(cum, ET * k / NCORES)))
    bnds.append(N)

    # --- per-core greedy tiling: whole nodes, <=EPT edges, <=SPT nodes ---
    core_tiles = []  # per core: list of (first_node, n_nodes)
    for k in range(NCORES):
        nb, ne = bnds[k], bnds[k + 1]
        tiles = []
        cur_first, cur_n, cur_e = nb, 0, 0
        for n in range(nb, ne):
            d = int(deg[n])
            if cur_n == SPT or cur_e + d > EPT:
                tiles.append((cur_first, cur_n))
                cur_first, cur_n, cur_e = n, 0, 0
            cur_n += 1
            cur_e += d
        tiles.append((cur_first, cur_n))
        core_tiles.append(tiles)

    lcm = np.lcm(KSUP, 512 // SPT)  # tiles multiple for chunking (=64)
    TMAX = max(len(t) for t in core_tiles)
    TMAX = int(-(-TMAX // lcm) * lcm)
    SLOTMAX = TMAX * SPT
    TROWS = 1 + NCORES * SLOTMAX
    assert TROWS <= 65536, TROWS
    TBASE = TROWS // 2

    # --- global slot-gid map (0 = poison row, then core-major slots) ---
    nodeslot = np.zeros(N, dtype=np.int64)
    for k in range(NCORES):
        for t, (first, nn) in enumerate(core_tiles[k]):
            ids = np.arange(first, first + nn)
            nodeslot[ids] = 1 + k * SLOTMAX + t * SPT + np.arange(nn)

    # --- per-core device input arrays ---
    per_core = []
    for k in range(NCORES):
        tiles = core_tiles[k]
        nb, ne = bnds[k], bnds[k + 1]
        nreal = ne - nb

        # padding sentinel TBASE: rebased int16 index 0 (must be >= 0 so a
        # trailing padding run is never "ignored" by the gather ucode)
        srcgid = np.full(TMAX * EPT, TBASE, dtype=np.int64)  # flat (t, e)
        OH = np.zeros((EPT, TMAX * SPT), dtype=np.float32)
        # stacked-transposed one-hots: per 16-tile group g, a [128,128] block
        # OHTS[8*(t%16) + s, (t//16)*128 + e] = OH[e, t*SPT + s]
        OHTS = np.zeros((128, (TMAX // 16) * EPT), dtype=np.float32)
        OH17T = np.zeros((NT, TMAX * EPT), dtype=np.float32)
        oh17 = np.zeros((NT, SLOTMAX), dtype=np.float32)

        for t, (first, nn) in enumerate(tiles):
            e0, e1 = node_ptr[first], node_ptr[first + nn]
            ne_t = e1 - e0
            assert ne_t <= EPT
            gsl = nodeslot[srcs[e0:e1]]
            dloc = (dsts[e0:e1] - first).astype(np.int64)
            tyl = x[srcs[e0:e1]]
            if ne_t == EPT and gsl[-1] < TBASE:
                # dma_gather ignores a trailing negative-index run: put a
                # non-negative-index edge last (edge order in a tile is free)
                pos = np.nonzero(gsl >= TBASE)[0]
                assert len(pos) > 0, "tile with all-negative rebased indices"
                j = pos[0]
                for arr in (gsl, dloc, tyl):
                    arr[j], arr[-1] = arr[-1].copy(), arr[j].copy()
            srcgid[t * EPT:t * EPT + ne_t] = gsl
            rows = np.arange(ne_t)
            OH[rows, t * SPT + dloc] = 1.0
            OHTS[(t % 16) * SPT + dloc, (t // 16) * EPT + rows] = 1.0
            OH17T[tyl, t * EPT + rows] = 1.0
            sl = t * SPT + np.arange(nn)
            oh17[x[first:first + nn], sl] = 1.0

        # int16 table indices rebased to TBASE, wrapped-16 for dma_gather:
        # flat idx position i lives at [16g + i%16, i//16] for every group g
        idx16 = (srcgid - TBASE).astype(np.int16)
        srcw = np.zeros((128, TMAX * EPT // 16), dtype=np.int16)
        i = np.arange(TMAX * EPT)
        for g in range(8):
            srcw[g * 16 + (i % 16), i // 16] = idx16

        MSK = np.zeros((128, 16), dtype=np.float32)
        MSK[np.arange(128), np.arange(128) // SPT] = 1.0

        npadvec = np.full((D, 1), SLOTMAX - nreal, dtype=np.float32)
        slot_node = np.full(SLOTMAX, -1, dtype=np.int64)
        for t, (first, nn) in enumerate(tiles):
            slot_node[t * SPT:t * SPT + nn] = np.arange(first, first + nn) - nb
        per_core.append(dict(
            srcw=srcw, OH=OH, OHTS=OHTS, OH17T=OH17T, oh17=oh17, MSK=MSK,
            npadvec=npadvec, nreal=nreal, nb=nb, ne=ne,
            slot_node=slot_node,
        ))

    # --- folded weights (shared across cores) ---
    f32 = np.float32
    Wcat1 = np.concatenate([_fold_ws(w1, as1), w1.astype(f32), _fold_ws(w1, ad1)], axis=1)
    Wcat2 = np.concatenate([_fold_ws(w2, as2), w2.astype(f32), _fold_ws(w2, ad2)], axis=1)
    # layer 3: records carry xin itself (identity block); scores fold w3/as3
    Wcat3 = np.concatenate([_fold_ws(w3, as3), np.eye(HID, dtype=f32), _fold_ws(w3, ad3)], axis=1)
    # W3stack[h*HID+c, o] = w3[c, h*D+o] / H   (mean over heads folded in)
    W3stack = (w3.reshape(HID, H, D).transpose(1, 0, 2).reshape(H * HID, D) / H).astype(f32)
    REP2 = np.zeros((H, HID), dtype=f32)
    REP2[np.arange(HID) // C1, np.arange(HID)] = 1.0
    # layer-3 agg: 3 psum tiles [100, cols]; tile j holds head 2j at rows 0:36
    # and head 2j+1 at rows 64:100 (psum base-partition must be 0/32/64)
    VA = 100
    REP3 = np.zeros((3, H, VA), dtype=f32)
    W3s = np.zeros((3, VA, D), dtype=f32)
    for j in range(3):
        REP3[j, 2 * j, 0:HID] = 1.0
        REP3[j, 2 * j + 1, 64:64 + HID] = 1.0
        W3s[j, 0:HID] = W3stack[(2 * j) * HID:(2 * j + 1) * HID]
        W3s[j, 64:64 + HID] = W3stack[(2 * j + 1) * HID:(2 * j + 2) * HID]
    zrow = np.zeros((1, RWT), dtype=f32)

    shared = dict(
        emb=emb.astype(f32), embT=emb.astype(f32).T.copy(),
        Wcat1=Wcat1, Wcat2=Wcat2, Wcat3=Wcat3,
        W3s0=W3s[0], W3s1=W3s[1], W3s2=W3s[2],
        r1=r1.astype(f32), r2=r2.astype(f32), r3=r3.astype(f32),
        b1=b1.astype(f32).reshape(-1, 1), b2=b2.astype(f32).reshape(-1, 1),
        b3=b3.astype(f32).reshape(-1, 1),
        REP2=REP2, REP30=REP3[0], REP31=REP3[1], REP32=REP3[2], zrow=zrow,
    )
    meta = dict(TMAX=TMAX, SLOTMAX=SLOTMAX, NMAXOUT=0,
                bnds=bnds, nreal=[pc['nreal'] for pc in per_core],
                slot_node=[pc['slot_node'] for pc in per_core])
    return per_core, shared, meta


def numpy_reference(x, edge_index, emb, w1, as1, ad1, b1, r1,
                    w2, as2, ad2, b2, r2, w3, as3, ad3, b3, r3):
    """Plain numpy port of reference.py for quick host validation."""
    def gat(xf, src, dst, W, a_s, a_d, b, r, concat):
        n = xf.shape[0]
        h, c = a_s.shape
        xs = (xf @ W).reshape(n, h, c)
        a_src = (xs * a_s).sum(-1)
        a_dst = (xs * a_d).sum(-1)
        e = a_src[src] + a_dst[dst]
        e = np.where(e > 0, e, NEG * e)
        ex = np.exp(e)
        s = np.zeros((n, h))
        np.add.at(s, dst, ex)
        alpha = ex / (s[dst] + 1e-16)
        out = np.zeros((n, h, c))
        np.add.at(out, dst, xs[src] * alpha[:, :, None])
        out = out.reshape(n, h * c) if concat else out.mean(1)
        return out + xf @ r + b

    hf = emb[np.asarray(x).astype(np.int64)]
    loops = np.arange(x.shape[0])
    src = np.concatenate([edge_index[0], loops])
    dst = np.concatenate([edge_index[1], loops])
    hf = np.maximum(gat(hf, src, dst, w1, as1, ad1, b1, r1, True), 0)
    hf = np.maximum(gat(hf, src, dst, w2, as2, ad2, b2, r2, True), 0)
    hf = gat(hf, src, dst, w3, as3, ad3, b3, r3, False)
    hf = hf - hf.max(0, keepdims=True)
    e = np.exp(hf)
    return (e / e.sum(0, keepdims=True)).astype(np.float32)


# ======================= device program =======================

import concourse.bass as bass
import concourse.tile as tile
from concourse import bacc, mybir
from concourse.tile import add_dep_helper

F32 = mybir.dt.float32
F32R = mybir.dt.float32r
I16 = mybir.dt.int16
BF16 = mybir.dt.bfloat16


def build_program(TMAX, SLOTMAX, NMAXOUT, D, HID, NT, n_cores=8, edge_dt=BF16,
                  debug_dump=False):
    NCH128 = SLOTMAX // 128
    NCH512 = SLOTMAX // 512
    TROWS = 1 + n_cores * SLOTMAX
    TBASE = TROWS // 2
    VA = 100                     # layer-3 agg tile rows (2 heads at 0:36, 64:100)
    NG16 = TMAX // 16            # 16-tile groups
    cores = list(range(n_cores))

    nc = bacc.Bacc("TRN2", target_bir_lowering=False, debug=False,
                   num_devices=n_cores)

    def din(name, shape, dt=F32):
        return nc.dram_tensor(name, list(shape), dt, kind="ExternalInput")

    srcw_d = din("srcw", [128, TMAX * EPT // 16], I16)
    oh_d = din("OH", [EPT, TMAX * SPT], edge_dt)
    ohts_d = din("OHTS", [128, NG16 * EPT], edge_dt)
    oh17t_d = din("OH17T", [NT, TMAX * EPT], edge_dt)
    oh17_d = din("oh17", [NT, SLOTMAX], edge_dt)
    msk_d = din("MSK", [128, 16], edge_dt)
    npad_d = din("npadvec", [D, 1])
    emb_d = din("emb", [NT, D], edge_dt)
    embt_d = din("embT", [D, NT], edge_dt)
    wcat_d = [din("Wcat1", [D, RW], edge_dt), din("Wcat2", [HID, RW], edge_dt),
              din("Wcat3", [HID, RW], edge_dt)]
    w3s_d = [din(f"W3s{j}", [VA, D], edge_dt) for j in range(3)]
    r_d = [din("r1", [D, HID], edge_dt), din("r2", [HID, HID], edge_dt),
           din("r3", [HID, D], edge_dt)]
    b_d = [din("b1", [HID, 1]), din("b2", [HID, 1]), din("b3", [D, 1])]
    rep2_d = din("REP2", [H, HID])
    rep3_d = [din(f"REP3{j}", [H, VA]) for j in range(3)]
    zb_d = din("ZB", [128, 32], edge_dt)
    zrow_d = din("zrow", [1, RWT], edge_dt)
    out_d = nc.dram_tensor("out", [D, SLOTMAX], F32, kind="ExternalOutput")
    if debug_dump:
        dbg1_d = nc.dram_tensor("dbg1", [HID, SLOTMAX], F32, kind="ExternalOutput")
        dbg2_d = nc.dram_tensor("dbg2", [HID, SLOTMAX], F32, kind="ExternalOutput")

    lsum_d = nc.dram_tensor("lsum", [D, 1], F32, kind="ExternalOutput")
    ag_in = nc.dram_tensor("ag_in", [SLOTMAX, RWT], edge_dt)
    table = nc.dram_tensor("table", [TROWS, RWT], edge_dt)

    with ExitStack() as ctx:
        tc = ctx.enter_context(tile.TileContext(nc))
        res = ctx.enter_context(tc.tile_pool(name="res", bufs=1))
        cst = ctx.enter_context(tc.tile_pool(name="cst", bufs=1))

        def load(pool, src, shape, dt=F32, tag=None):
            t = pool.tile(list(shape), dt, tag=tag)
            nc.sync.dma_start(out=t[:], in_=src[:])
            return t

        msk_sb = load(cst, msk_d, [128, 16], edge_dt, tag="msk")
        npad_sb = load(cst, npad_d, [D, 1], tag="npad")
        emb_sb = load(cst, emb_d, [NT, D], edge_dt, tag="emb")
        embt_sb = load(cst, embt_d, [D, NT], edge_dt, tag="embt")
        wcat_sb = [load(cst, wcat_d[0], [D, RW], edge_dt, tag="wc1"),
                   load(cst, wcat_d[1], [HID, RW], edge_dt, tag="wc2"),
                   load(cst, wcat_d[2], [HID, RW], edge_dt, tag="wc3")]
        w3s_sb = [load(cst, w3s_d[j], [VA, D], edge_dt, tag=f"w3s{j}")
                  for j in range(3)]
        r_sb = [load(cst, r_d[0], [D, HID], edge_dt, tag="r1"),
                load(cst, r_d[1], [HID, HID], edge_dt, tag="r2"),
                load(cst, r_d[2], [HID, D], edge_dt, tag="r3")]
        b_sb = [load(cst, b_d[0], [HID, 1], tag="b1"),
                load(cst, b_d[1], [HID, 1], tag="b2"),
                load(cst, b_d[2], [D, 1], tag="b3")]
        rep2_sb = load(cst, rep2_d, [H, HID], tag="rep2")
        rep3_sb = [load(cst, rep3_d[j], [H, VA], tag=f"rep3{j}")
                   for j in range(3)]
        zb_sb = load(cst, zb_d, [128, 32], edge_dt, tag="zb")
        zw = nc.sync.dma_start(out=table[0:1, :], in_=zrow_d[:])

        # ---- t17 = per-type layer-1 records [NT, RW] ----
        t17_sb = cst.tile([NT, RW], edge_dt, tag="t17")
        with tc.tile_pool(name="p17", bufs=1, space="PSUM") as p17:
            ps = p17.tile([NT, RW], F32, space="PSUM", tag="ps")
            nc.tensor.matmul(out=ps[:], lhsT=embt_sb[:], rhs=wcat_sb[0][:],
                             start=True, stop=True)
            nc.vector.tensor_copy(t17_sb[:], ps[:])

        # ---- h0T = emb.T @ onehot17, fused with layer-0 records
        # (rec[slot] = t17[type(slot)] needs only oh17 + t17, not hT0) ----
        hT0 = res.tile([D, SLOTMAX], edge_dt, tag="h0")
        recS = res.tile([128, NCH128 * RW], edge_dt, tag="recS")
        with tc.tile_pool(name="p0", bufs=2) as p0, \
             tc.tile_pool(name="ps0", bufs=2, space="PSUM") as ps0:
            for c in range(NCH512):
                ohc = p0.tile([NT, 512], edge_dt, tag="ohc")
                nc.sync.dma_start(out=ohc[:], in_=oh17_d[:, c * 512:(c + 1) * 512])
                psn = ps0.tile([128, 4 * RW], F32, space="PSUM", tag="psn")
                for j in range(4):
                    nc.tensor.matmul(
                        out=psn[:, j * RW:(j + 1) * RW],
                        lhsT=ohc[:, j * 128:(j + 1) * 128],
                        rhs=t17_sb[:], start=True, stop=True)
                nc.scalar.copy(out=recS[:, c * 4 * RW:(c + 1) * 4 * RW],
                               in_=psn[:])
                ps = ps0.tile([D, 512], F32, space="PSUM", tag="ps")
                nc.tensor.matmul(out=ps[:], lhsT=emb_sb[:], rhs=ohc[:],
                                 start=True, stop=True)
                nc.scalar.copy(out=hT0[:, c * 512:(c + 1) * 512], in_=ps[:])

        oh_sb = load(res, oh_d, [EPT, TMAX * SPT], edge_dt, tag="oh")
        ohts_sb = load(res, ohts_d, [128, NG16 * EPT], edge_dt, tag="ohts")

        hT1 = res.tile([HID, SLOTMAX], edge_dt, tag="h1")
        hT2 = res.tile([HID, SLOTMAX], edge_dt, tag="h2")
        agg3 = [res.tile([VA, SLOTMAX], edge_dt, tag=f"agg{j}",
                         name=f"agg{j}") for j in range(3)]
        out3T = res.tile([D, SLOTMAX], F32, tag="o3")
        hins = [hT0, hT1, hT2]
        houts = [hT1, hT2, None]
        prev_cc = None
        prev_readers = []

        srcw = load(res, srcw_d, [128, TMAX * EPT // 16], I16, tag="srcw")
        lsums = res.tile([D, NCH512], F32, tag="lsums")
        for l in range(3):
            hin = hins[l]

            # layer-0 records were fused into the hT0 loop; later layers'
            # records are emitted inside the previous layer's super loop
            if l > 0:
                # records were computed during layer l-1; ship them out
                wdma = nc.sync.dma_start(
                    out=ag_in[:, 0:RW].rearrange("(c p) w -> p c w", p=128),
                    in_=recS[:].rearrange("p (c w) -> p c w", w=RW))
                if prev_cc is not None:
                    add_dep_helper(wdma.ins, prev_cc.ins,
                                   reason="ag_in WAR vs previous AllGather")
                if n_cores == 1:
                    cc = nc.sync.dma_start(out=table[1:, 0:RW], in_=ag_in[:, 0:RW])
                else:
                    cc = nc.gpsimd.collective_compute(
                        "AllGather", mybir.AluOpType.bypass,
                        replica_groups=[cores],
                        ins=[ag_in[:]], outs=[table[1:, :]],
                    )
                for rd in prev_readers:
                    add_dep_helper(cc.ins, rd.ins,
                                   reason="table WAR vs previous layer gathers")
                prev_cc = cc
                prev_readers = []

            # ---- edge phase, with P4 / next-layer P1 interleaved ----
            ks = KSUP
            nsup = TMAX // ks
            cols = ks * SPT                      # psum cols per super (256)
            with tc.tile_pool(name=f"ed{l}", bufs=2) as wp, \
                 tc.tile_pool(name=f"edp{l}", bufs=2, space="PSUM") as pp, \
                 tc.tile_pool(name=f"adp{l}", bufs=1, space="PSUM") as adp, \
                 tc.tile_pool(name=f"fin{l}", bufs=(2 if l == 1 else 1),
                              space="PSUM") as fpp, \
                 tc.tile_pool(name=f"rec{l}", bufs=1, space="PSUM") as rpp:
                for g in range(nsup):
                    t0 = g * ks
                    Rg = wp.tile([EPT, ks * RWT], edge_dt, tag="Rg")
                    R3 = Rg[:].rearrange("p (k e) -> p k e", e=RWT)
                    if l == 0:
                        o17 = wp.tile([NT, ks * EPT], edge_dt, tag="o17")
                        nc.sync.dma_start(
                            out=o17[:],
                            in_=oh17t_d[:, t0 * EPT:(t0 + ks) * EPT])
                        for k8 in range(ks // 8):
                            psR = adp.tile([128, 8 * RW], F32, space="PSUM",
                                           tag="psR", bufs=2)
                            for j in range(8):
                                nc.tensor.matmul(
                                    out=psR[:, j * RW:(j + 1) * RW],
                                    lhsT=o17[:, (k8 * 8 + j) * EPT:
                                             (k8 * 8 + j + 1) * EPT],
                                    rhs=t17_sb[:], start=True, stop=True)
                            nc.scalar.copy(
                                out=R3[:, k8 * 8:(k8 + 1) * 8, 0:RW],
                                in_=psR[:].rearrange("p (k w) -> p k w", w=RW))
                    else:
                        # SWDGE ring holds 1024 descriptors -> 1024 idxs max
                        # per dma_gather; 4 sub-gathers cover the 32 tiles
                        for j4 in range(ks // 8):
                            i0 = (g * ks + j4 * 8) * EPT // 16
                            gi = nc.gpsimd.dma_gather(
                                out_ap=R3[:, j4 * 8:(j4 + 1) * 8, :],
                                in_ap=table[TBASE:, :],
                                idxs_ap=srcw[:, i0:i0 + 8 * EPT // 16],
                                num_idxs=8 * EPT, num_idxs_reg=8 * EPT,
                                elem_size=RWT)
                            add_dep_helper(gi.ins, prev_cc.ins,
                                           reason="gather RAW AllGather")
                            prev_readers.append(gi)

                    # a_dst expansion: block-diagonal stacked-OHT matmuls
                    # (one psum bank also carries l2's psS in cols 192:448)
                    psADb = adp.tile([EPT, ks * H + KSUP * SPT], F32,
                                     space="PSUM", tag="psAD")
                    for j in range(ks // 16):
                        cb = 2 * g + j       # 128-slot chunk index
                        bd = wp.tile([128, 16 * H], edge_dt, tag="bd")
                        nc.vector.tensor_tensor(
                            out=bd[:].rearrange("p (k h) -> p k h", h=H),
                            in0=recS[:, cb * RW + RW - H:(cb + 1) * RW][:, None, :]
                                .to_broadcast([128, 16, H]),
                            in1=msk_sb[:][:, :, None].to_broadcast([128, 16, H]),
                            op=mybir.AluOpType.mult)
                        nc.tensor.matmul(
                            out=psADb[:, j * 16 * H:(j + 1) * 16 * H],
                            lhsT=ohts_sb[:, cb * EPT:(cb + 1) * EPT],
                            rhs=bd[:], start=True, stop=True)

                    esc = wp.tile([EPT, ks * H], F32, tag="esc")
                    nc.vector.tensor_tensor(
                        out=esc[:], in0=R3[:, :, 0:H],
                        in1=psADb[:, 0:ks * H], op=mybir.AluOpType.add)
                    nc.vector.scalar_tensor_tensor(
                        out=esc[:], in0=esc[:], scalar=NEG, in1=esc[:],
                        op0=mybir.AluOpType.mult, op1=mybir.AluOpType.max)

                    if l < 2:
                        # per-tile lhsT layout: xs*ex at rows 0:36, gap, ex at
                        # rows 64:70 -> psum reads start at partitions 0 / 64
                        lw = 70
                        RHS = wp.tile([EPT, ks * lw], edge_dt, tag="RHS")
                        S3 = RHS[:].rearrange("p (k e) -> p k e", e=lw)
                        nc.scalar.activation(
                            out=S3[:, :, 64:64 + H],
                            in_=esc[:].rearrange("p (k h) -> p k h", h=H),
                            func=mybir.ActivationFunctionType.Exp)
                        ex_rep = S3[:, :, 64:64 + H][:, :, :, None].to_broadcast(
                            [EPT, ks, H, C1])
                        nc.vector.tensor_tensor(
                            out=S3[:, :, 0:HID].rearrange(
                                "p k (h c) -> p k h c", h=H),
                            in0=R3[:, :, H:RW - H].rearrange(
                                "p k (h c) -> p k h c", h=H),
                            in1=ex_rep, op=mybir.AluOpType.mult)
                        psSV = pp.tile([lw, cols], F32, space="PSUM", tag="psSV")
                        for k in range(ks):
                            nc.tensor.matmul(
                                out=psSV[:, k * SPT:(k + 1) * SPT],
                                lhsT=RHS[:, k * lw:(k + 1) * lw],
                                rhs=oh_sb[:, (t0 + k) * SPT:(t0 + k + 1) * SPT],
                                start=True, stop=True)
                        rs = wp.tile([H, cols], F32, tag="rs")
                        nc.vector.tensor_scalar_add(out=rs[:],
                                                    in0=psSV[64:64 + H, :],
                                                    scalar1=1e-16)
                        nc.vector.reciprocal(out=rs[:], in_=rs[:])
                        csl = slice(g * cols, (g + 1) * cols)
                        ps2 = pp.tile([HID, cols], F32, space="PSUM", tag="ps2",
                                      bufs=2 if l == 1 else 1)
                        nc.tensor.matmul(out=ps2[:],
                                         lhsT=rep2_sb[:], rhs=rs[:],
                                         start=True, stop=True)
                        nc.scalar.copy(out=houts[l][:, csl],
                                       in_=psSV[0:HID, :])
                        nc.vector.tensor_tensor(
                            out=houts[l][:, csl], in0=houts[l][:, csl],
                            in1=ps2[:], op=mybir.AluOpType.mult)
                    else:
                        EX = wp.tile([EPT, ks * H], edge_dt, tag="EX")
                        nc.scalar.activation(
                            out=EX[:], in_=esc[:],
                            func=mybir.ActivationFunctionType.Exp)
                        OHx = wp.tile([EPT, ks * H * SPT], edge_dt, tag="OHx")
                        nc.vector.tensor_tensor(
                            out=OHx[:].rearrange("p (k h v) -> p k h v", h=H,
                                                 v=SPT),
                            in0=oh_sb[:, t0 * SPT:(t0 + ks) * SPT].rearrange(
                                "p (k v) -> p k v", v=SPT)[:, :, None, :]
                                .to_broadcast([EPT, ks, H, SPT]),
                            in1=EX[:].rearrange("p (k h) -> p k h", h=H)
                                [:, :, :, None].to_broadcast([EPT, ks, H, SPT]),
                            op=mybir.AluOpType.mult)
                        psS = psADb[0:H, ks * H:ks * H + cols]
                        # PH[0]/PH[1] packed into one 2KB psum bank
                        PH01 = pp.tile([VA, 2 * cols], F32, space="PSUM",
                                       tag="psH01")
                        PH2 = pp.tile([VA, cols], F32, space="PSUM", tag="psH2")
                        PH = [PH01[:, 0:cols], PH01[:, cols:2 * cols], PH2[:]]
                        # rows 36:64 are never head-written; zero them so
                        # the agg multiply stays finite
                        nc.tensor.matmul(
                            out=PH01[32:64, :], lhsT=zb_sb[:],
                            rhs=oh_sb[:, 0:2 * cols], start=True, stop=True)
                        nc.tensor.matmul(
                            out=PH2[32:64, :], lhsT=zb_sb[:],
                            rhs=oh_sb[:, 0:cols], start=True, stop=True)
                        for k in range(ks):
                            nc.tensor.matmul(
                                out=psS[:, k * SPT:(k + 1) * SPT],
                                lhsT=EX[:, k * H:(k + 1) * H],
                                rhs=oh_sb[:, (t0 + k) * SPT:(t0 + k + 1) * SPT],
                                start=True, stop=True)
                            xsk = Rg[:, k * RWT + H:k * RWT + RW - H]
                            for h in range(H):
                                rb = (h % 2) * 64
                                PHj = PH[h // 2]
                                nc.tensor.matmul(
                                    out=PHj[rb:rb + HID,
                                            k * SPT:(k + 1) * SPT],
                                    lhsT=xsk,
                                    rhs=OHx[:, (k * H + h) * SPT:
                                            (k * H + h + 1) * SPT],
                                    start=True, stop=True)
                        rs = wp.tile([H, cols], F32, tag="rs")
                        nc.vector.tensor_scalar_add(out=rs[:], in0=psS,
                                                    scalar1=1e-16)
                        nc.vector.reciprocal(out=rs[:], in_=rs[:])
                        csl = slice(g * cols, (g + 1) * cols)
                        ps2p01 = pp.tile([VA, 2 * cols], F32, space="PSUM",
                                         tag="ps2p01", bufs=1)
                        ps2p2 = pp.tile([VA, cols], F32, space="PSUM",
                                        tag="ps2p2", bufs=1)
                        PS2 = [ps2p01[:, 0:cols], ps2p01[:, cols:2 * cols],
                               ps2p2[:]]
                        for j in range(3):
                            nc.tensor.matmul(out=PS2[j],
                                             lhsT=rep3_sb[j][:], rhs=rs[:],
                                             start=True, stop=True)
                            nc.scalar.copy(out=agg3[j][:, csl], in_=PH[j])
                            nc.vector.tensor_tensor(
                                out=agg3[j][:, csl], in0=agg3[j][:, csl],
                                in1=PS2[j], op=mybir.AluOpType.mult)

                    # ---- interleaved P4 (+ next-layer P1 / output chunk) ----
                    if g % 2 == 1:
                        c4 = g // 2
                        csl = slice(c4 * 512, (c4 + 1) * 512)
                        if l < 2:
                            ps = fpp.tile([HID, 512], F32, space="PSUM",
                                          tag="ps")
                            nc.tensor.matmul(out=ps[:], lhsT=r_sb[l][:],
                                             rhs=hin[:, csl],
                                             start=True, stop=True)
                            nc.vector.tensor_tensor(
                                out=houts[l][:, csl], in0=houts[l][:, csl],
                                in1=ps[:], op=mybir.AluOpType.add)
                            nc.scalar.activation(
                                out=houts[l][:, csl], in_=houts[l][:, csl],
                                func=mybir.ActivationFunctionType.Relu,
                                bias=b_sb[l][:])
                            # next layer's records for these 512 slots
                            psn = rpp.tile([128, 4 * RW], F32, space="PSUM",
                                           tag="psn")
                            for j in range(4):
                                c = c4 * 4 + j
                                nc.tensor.matmul(
                                    out=psn[:, j * RW:(j + 1) * RW],
                                    lhsT=houts[l][:, c * 128:(c + 1) * 128],
                                    rhs=wcat_sb[l + 1][:],
                                    start=True, stop=True)
                            nc.scalar.copy(
                                out=recS[:, c4 * 4 * RW:(c4 + 1) * 4 * RW],
                                in_=psn[:])
                        else:
                            ps = fpp.tile([D, 512], F32, space="PSUM",
                                          tag="ps64")
                            nc.tensor.matmul(out=ps[:], lhsT=w3s_sb[0][:],
                                             rhs=agg3[0][:, csl],
                                             start=True, stop=False)
                            nc.tensor.matmul(out=ps[:], lhsT=w3s_sb[1][:],
                                             rhs=agg3[1][:, csl],
                                             start=False, stop=False)
                            nc.tensor.matmul(out=ps[:], lhsT=w3s_sb[2][:],
                                             rhs=agg3[2][:, csl],
                                             start=False, stop=False)
                            nc.tensor.matmul(out=ps[:], lhsT=r_sb[2][:],
                                             rhs=hin[:, csl],
                                             start=False, stop=True)
                            # exp without max-subtraction (outputs are O(1));
                            # normalization happens on host from lsum partials
                            nc.vector.tensor_scalar_add(out=out3T[:, csl],
                                                        in0=ps[:],
                                                        scalar1=b_sb[2][:])
                            nc.scalar.activation(
                                out=out3T[:, csl], in_=out3T[:, csl],
                                func=mybir.ActivationFunctionType.Exp)
                            nc.vector.tensor_reduce(
                                out=lsums[:, c4:c4 + 1], in_=out3T[:, csl],
                                axis=mybir.AxisListType.X,
                                op=mybir.AluOpType.add)
                            nc.sync.dma_start(out=out_d[:, csl],
                                              in_=out3T[:, csl])

            if debug_dump:
                if l == 0:
                    nc.sync.dma_start(out=dbg1_d[:], in_=hT1[:])
                elif l == 1:
                    nc.sync.dma_start(out=dbg2_d[:], in_=hT2[:])

        # ---- final: per-core softmax denominator partials to host ----
        with tc.tile_pool(name="sm", bufs=1) as sp:
            lsum = sp.tile([D, 1], F32, tag="lsum")
            nc.vector.tensor_reduce(out=lsum[:], in_=lsums[:],
                                    axis=mybir.AxisListType.X,
                                    op=mybir.AluOpType.add)
            # pad columns hold exp(0 + b3) = 1 each; subtract npad of them
            nc.vector.tensor_sub(out=lsum[:], in0=lsum[:], in1=npad_sb[:])
            nc.sync.dma_start(out=lsum_d[:], in_=lsum[:])

    nc.compile()
    return nc


# ======================= runner =======================
_CACHE = {}


def _make_in_maps(per_core, shared):
    ebf = ml_dtypes.bfloat16
    in_maps = []
    for pc in per_core:
        in_maps.append(dict(
            srcw=pc['srcw'],
            OH=pc['OH'].astype(ebf), OHTS=pc['OHTS'].astype(ebf),
            OH17T=pc['OH17T'].astype(ebf), oh17=pc['oh17'].astype(ebf),
            MSK=pc['MSK'].astype(ebf), npadvec=pc['npadvec'],
            emb=shared['emb'].astype(ebf), embT=shared['embT'].astype(ebf),
            Wcat1=shared['Wcat1'].astype(ebf), Wcat2=shared['Wcat2'].astype(ebf),
            Wcat3=shared['Wcat3'].astype(ebf),
            W3s0=shared['W3s0'].astype(ebf), W3s1=shared['W3s1'].astype(ebf),
            W3s2=shared['W3s2'].astype(ebf),
            r1=shared['r1'].astype(ebf), r2=shared['r2'].astype(ebf),
            r3=shared['r3'].astype(ebf), b1=shared['b1'], b2=shared['b2'],
            b3=shared['b3'], REP2=shared['REP2'], REP30=shared['REP30'],
            REP31=shared['REP31'], REP32=shared['REP32'],
            ZB=np.zeros((128, 32), ebf), zrow=shared['zrow'].astype(ebf),
        ))
    return in_maps


def kernel(x, edge_index, edge_attr=None, **w):
    """Full inputs in, full [50000, 64] float32 softmax output out."""
    from concourse.bass_utils import run_bass_kernel_spmd
    args = dict(x=x, edge_index=edge_index)
    for k in ('emb', 'w1', 'as1', 'ad1', 'b1', 'r1', 'w2', 'as2', 'ad2', 'b2',
              'r2', 'w3', 'as3', 'ad3', 'b3', 'r3'):
        args[k] = np.asarray(w[k])
    per_core, shared, meta = host_prep(**args)
    key = (meta['TMAX'], meta['SLOTMAX'],
           shared['emb'].shape, shared['r2'].shape)
    if key not in _CACHE:
        _CACHE[key] = build_program(
            meta['TMAX'], meta['SLOTMAX'], meta['NMAXOUT'],
            shared['emb'].shape[1], shared['r2'].shape[0],
            shared['emb'].shape[0])
    nc = _CACHE[key]
    in_maps = _make_in_maps(per_core, shared)
    res = run_bass_kernel_spmd(nc, in_maps, list(range(NCORES)))
    D = shared['emb'].shape[1]
    N = meta['bnds'][-1]
    gsum = np.zeros(D, np.float64)
    for k in range(NCORES):
        gsum += res.results[k]['lsum'][:, 0].astype(np.float64)
    out = np.zeros((N, D), np.float32)
    for k in range(NCORES):
        nb = meta['bnds'][k]
        sn = meta['slot_node'][k]
        real = sn >= 0
        out[nb + sn[real]] = (res.results[k]['out'].T[real] /
                              gsum[None, :]).astype(np.float32)
    return out
